# revision 1
# baseline (speedup 1.0000x reference)
"""Graphormer attention head (block-diagonal, 32 graphs x 128 nodes) on 8 trn2 cores.

Sharding: graphs (row blocks of 128) across cores, 4 graphs per core.
Each core gets its 512 rows of x / b / edge_encoding plus replicated
projection weights.  b/edge slices are column-rotated on the host by
-512*core so the diagonal block of every core lands at the same columns,
letting all 8 cores run one SPMD program.

Math per row block g (128 rows, full 4096 columns), matching the reference:
    scores = (QK^T*scale + b + e) in-block ; (b + e) * -1e6 off-block
    softmax over the full row, then in-block probs @ V.
Off-block handling is fused: one DVE tensor_tensor_reduce computes
t = b+e and its row-min (-> off-block row max = -1e6*min), one ScalarE
activation computes exp(-1e6*t - rowmax) with the row-sum accumulated
for the softmax denominator.
"""

import math
import os

import numpy as np

os.environ.setdefault("MYCRO_LOCAL_CACHE", "1")

N = 4096
DIN = 512
DQ = 512
NCORES = 8
RPC = N // NCORES          # rows per core = 512
GPC = 4                    # graphs per core
M = 128                    # graph size
IC = DIN // 128            # 4 input chunks
OC = DQ // 128             # 4 output chunks
NEG = -1000000.0
FMAX = 3.0e38

_cache = {}


def _build_bass():
    import concourse.mybir as mybir
    import concourse.tile as tile
    from concourse import bacc

    f32 = mybir.dt.float32
    bf16 = mybir.dt.bfloat16
    Alu = mybir.AluOpType
    Act = mybir.ActivationFunctionType
    Axis = mybir.AxisListType

    nc = bacc.Bacc("TRN2", target_bir_lowering=False)

    xT = nc.dram_tensor("xT", [IC, 128, RPC], bf16, kind="ExternalInput")
    wqT = nc.dram_tensor("wqT", [IC, 128, DQ], bf16, kind="ExternalInput")
    wkT = nc.dram_tensor("wkT", [IC, 128, DQ], bf16, kind="ExternalInput")
    wvT = nc.dram_tensor("wvT", [IC, 128, DQ], bf16, kind="ExternalInput")
    bqs = nc.dram_tensor("bqs", [1, DQ], bf16, kind="ExternalInput")
    bks = nc.dram_tensor("bks", [1, DQ], bf16, kind="ExternalInput")
    bvr = nc.dram_tensor("bvr", [1, DQ], bf16, kind="ExternalInput")
    ones = nc.dram_tensor("ones", [1, DQ], bf16, kind="ExternalInput")
    ident = nc.dram_tensor("ident", [128, 128], bf16, kind="ExternalInput")
    b_in = nc.dram_tensor("b_in", [GPC, 128, N], f32, kind="ExternalInput")
    e_in = nc.dram_tensor("e_in", [GPC, 128, N], f32, kind="ExternalInput")
    out = nc.dram_tensor("out", [RPC, DQ], f32, kind="ExternalOutput")

    with tile.TileContext(nc) as tc:
        with (
            tc.tile_pool(name="const", bufs=1) as const,
            tc.tile_pool(name="qkv", bufs=1) as qkv,
            tc.tile_pool(name="big", bufs=4) as big,
            tc.tile_pool(name="small", bufs=4) as small,
            tc.tile_pool(name="stat", bufs=8) as stat,
            tc.tile_pool(name="psA", bufs=2, space="PSUM") as psA,
            tc.tile_pool(name="psS", bufs=2, space="PSUM") as psS,
            tc.tile_pool(name="psT", bufs=2, space="PSUM") as psT,
            tc.tile_pool(name="psO", bufs=2, space="PSUM") as psO,
        ):
            # ---- load constants ----
            xT_t, wqT_t, wkT_t, wvT_t = [], [], [], []
            for i in range(IC):
                t = const.tile([128, RPC], bf16, tag=f"xT{i}")
                nc.sync.dma_start(out=t[:], in_=xT[i])
                xT_t.append(t)
            for name, dram, lst in (
                ("wq", wqT, wqT_t), ("wk", wkT, wkT_t), ("wv", wvT, wvT_t)
            ):
                for i in range(IC):
                    t = const.tile([128, DQ], bf16, tag=f"{name}{i}")
                    nc.sync.dma_start(out=t[:], in_=dram[i])
                    lst.append(t)
            bq_t = const.tile([1, DQ], bf16, tag="bq")
            nc.sync.dma_start(out=bq_t[:], in_=bqs[:])
            bk_t = const.tile([1, DQ], bf16, tag="bk")
            nc.sync.dma_start(out=bk_t[:], in_=bks[:])
            bv_t = const.tile([1, DQ], bf16, tag="bv")
            nc.sync.dma_start(out=bv_t[:], in_=bvr[:])
            ones_t = const.tile([1, DQ], bf16, tag="ones")
            nc.sync.dma_start(out=ones_t[:], in_=ones[:])
            id_t = const.tile([128, 128], bf16, tag="ident")
            nc.sync.dma_start(out=id_t[:], in_=ident[:])

            # ---- projections ----
            # QT[o, r] = (Wq*scale) @ x^T + bq*scale ; KT likewise; V[r, o] natural.
            qT_t, kT_t, v_t = [], [], []
            for oc in range(OC):
                ps = psA.tile([128, RPC], f32)
                for ic in range(IC):
                    nc.tensor.matmul(
                        ps[:], wqT_t[ic][:, oc * 128:(oc + 1) * 128], xT_t[ic][:],
                        start=(ic == 0), stop=False,
                    )
                nc.tensor.matmul(ps[:], bq_t[:1, oc * 128:(oc + 1) * 128],
                                 ones_t[:1, :RPC], start=False, stop=True)
                t = qkv.tile([128, RPC], bf16, tag=f"qT{oc}")
                nc.scalar.copy(t[:], ps[:])
                qT_t.append(t)
            for oc in range(OC):
                ps = psA.tile([128, RPC], f32)
                for ic in range(IC):
                    nc.tensor.matmul(
                        ps[:], wkT_t[ic][:, oc * 128:(oc + 1) * 128], xT_t[ic][:],
                        start=(ic == 0), stop=False,
                    )
                nc.tensor.matmul(ps[:], bk_t[:1, oc * 128:(oc + 1) * 128],
                                 ones_t[:1, :RPC], start=False, stop=True)
                t = qkv.tile([128, RPC], bf16, tag=f"kT{oc}")
                nc.scalar.copy(t[:], ps[:])
                kT_t.append(t)
            for rc in range(GPC):
                ps = psA.tile([128, DQ], f32)
                for ic in range(IC):
                    nc.tensor.matmul(
                        ps[:], xT_t[ic][:, rc * 128:(rc + 1) * 128], wvT_t[ic][:],
                        start=(ic == 0), stop=False,
                    )
                nc.tensor.matmul(ps[:], ones_t[:1, :128], bv_t[:1, :],
                                 start=False, stop=True)
                t = qkv.tile([128, DQ], bf16, tag=f"v{rc}")
                nc.scalar.copy(t[:], ps[:])
                v_t.append(t)

            # ---- per-graph attention ----
            for g in range(GPC):
                w0 = g * M
                w1 = w0 + M
                b_t = big.tile([128, N], f32, tag="b")
                e_t = big.tile([128, N], f32, tag="e")
                nc.sync.dma_start(out=b_t[:], in_=b_in[g])
                nc.sync.dma_start(out=e_t[:], in_=e_in[g])

                # S' = (Q*scale) @ K^T for this graph's 128x128 block
                sps = psS.tile([128, M], f32)
                for oc in range(OC):
                    nc.tensor.matmul(
                        sps[:], qT_t[oc][:, w0:w1], kT_t[oc][:, w0:w1],
                        start=(oc == 0), stop=(oc == OC - 1),
                    )

                # t = b+e (in place over b), then off-block row-min per side
                nc.vector.tensor_add(b_t[:], b_t[:], e_t[:])
                mins = []
                if w0 > 0:
                    mn = stat.tile([128, 1], f32)
                    nc.vector.tensor_reduce(mn[:], b_t[:, 0:w0],
                                            axis=Axis.X, op=Alu.min)
                    mins.append(mn)
                if w1 < N:
                    mn = stat.tile([128, 1], f32)
                    nc.vector.tensor_reduce(mn[:], b_t[:, w1:N],
                                            axis=Axis.X, op=Alu.min)
                    mins.append(mn)

                # window: s_in = S' + (b+e)_win, then row-max
                s_in = small.tile([128, M], f32, tag="sin")
                nc.vector.tensor_add(s_in[:], sps[:], b_t[:, w0:w1])
                mx_in = stat.tile([128, 1], f32)
                nc.vector.tensor_reduce(mx_in[:], s_in[:], axis=Axis.X, op=Alu.max)

                # rowmax = max(-1e6 * min_off, max_in); negM = -rowmax
                m_off = stat.tile([128, 1], f32)
                if len(mins) == 2:
                    nc.vector.tensor_tensor(m_off[:], mins[0][:], mins[1][:],
                                            op=Alu.min)
                else:
                    nc.vector.tensor_copy(m_off[:], mins[0][:])
                mx_off = stat.tile([128, 1], f32)
                nc.vector.tensor_scalar_mul(mx_off[:], m_off[:], NEG)
                rowmax = stat.tile([128, 1], f32)
                nc.vector.tensor_tensor(rowmax[:], mx_off[:], mx_in[:], op=Alu.max)
                negM = stat.tile([128, 1], f32)
                nc.vector.tensor_scalar_mul(negM[:], rowmax[:], -1.0)

                # exp passes (into e tile), denominators accumulated
                sums = []
                if w0 > 0:
                    sm = stat.tile([128, 1], f32)
                    nc.scalar.activation(e_t[:, 0:w0], b_t[:, 0:w0], Act.Exp,
                                         bias=negM[:], scale=NEG, accum_out=sm[:])
                    sums.append(sm)
                if w1 < N:
                    sm = stat.tile([128, 1], f32)
                    nc.scalar.activation(e_t[:, w1:N], b_t[:, w1:N], Act.Exp,
                                         bias=negM[:], scale=NEG, accum_out=sm[:])
                    sums.append(sm)
                smw = stat.tile([128, 1], f32)
                nc.scalar.activation(e_t[:, w0:w1], s_in[:], Act.Exp,
                                     bias=negM[:], scale=1.0, accum_out=smw[:])

                denom = stat.tile([128, 1], f32)
                nc.vector.tensor_tensor(denom[:], sums[0][:], smw[:], op=Alu.add)
                if len(sums) == 2:
                    nc.vector.tensor_tensor(denom[:], denom[:], sums[1][:],
                                            op=Alu.add)
                rden = stat.tile([128, 1], f32)
                nc.vector.reciprocal(rden[:], denom[:])

                # P = exp_win / denom (bf16), transpose on PE, P^T @ V
                p_t = small.tile([128, M], bf16, tag="p")
                nc.vector.tensor_scalar_mul(p_t[:], e_t[:, w0:w1], rden[:])
                ptp = psT.tile([128, M], bf16)
                nc.tensor.transpose(ptp[:], p_t[:], id_t[:])
                pt_t = small.tile([128, M], bf16, tag="pt")
                nc.scalar.copy(pt_t[:], ptp[:])
                ops = psO.tile([128, DQ], f32)
                nc.tensor.matmul(ops[:], pt_t[:], v_t[g][:], start=True, stop=True)
                o_t = small.tile([128, DQ], f32, tag="o")
                nc.scalar.copy(o_t[:], ops[:])
                nc.sync.dma_start(out=out[g * M:(g + 1) * M, :], in_=o_t[:])

    nc.compile()
    return nc


def _get_bass():
    if "nc" not in _cache:
        _cache["nc"] = _build_bass()
    return _cache["nc"]


def _prepare_in_maps(x, b, e, Wq, bq, Wk, bk, Wv, bv):
    import ml_dtypes

    bf16 = ml_dtypes.bfloat16
    scale = 1.0 / math.sqrt(DQ)

    wq_s = (Wq.astype(np.float32) * scale)
    bq_s = (bq.astype(np.float32) * scale)
    wqT = np.ascontiguousarray(wq_s.T.reshape(IC, 128, DQ).astype(bf16))
    wkT = np.ascontiguousarray(Wk.T.reshape(IC, 128, DQ).astype(bf16))
    wvT = np.ascontiguousarray(Wv.T.reshape(IC, 128, DQ).astype(bf16))
    bqs = bq_s.reshape(1, DQ).astype(bf16)
    bks = bk.astype(np.float32).reshape(1, DQ).astype(bf16)
    bvr = bv.astype(np.float32).reshape(1, DQ).astype(bf16)
    ones = np.ones((1, DQ), dtype=bf16)
    ident = np.eye(128, dtype=bf16)

    in_maps = []
    for c in range(NCORES):
        rows = slice(c * RPC, (c + 1) * RPC)
        xT_c = np.ascontiguousarray(
            x[rows].astype(np.float32).T.reshape(IC, 128, RPC).astype(bf16))
        b_c = np.ascontiguousarray(
            np.roll(b[rows], -c * RPC, axis=1).reshape(GPC, 128, N)
        ).astype(np.float32)
        e_c = np.ascontiguousarray(
            np.roll(e[rows], -c * RPC, axis=1).reshape(GPC, 128, N)
        ).astype(np.float32)
        in_maps.append({
            "xT": xT_c, "wqT": wqT, "wkT": wkT, "wvT": wvT,
            "bqs": bqs, "bks": bks, "bvr": bvr, "ones": ones,
            "ident": ident, "b_in": b_c, "e_in": e_c,
        })
    return in_maps


def _reference_numpy(x, b, e, ptr, Wq, bq, Wk, bk, Wv, bv):
    """Fallback for unexpected ptr layouts: straight fp32 numpy port."""
    n = x.shape[0]
    graph_id = np.searchsorted(ptr, np.arange(n), side="right") - 1
    mask = graph_id[:, None] == graph_id[None, :]
    q = x @ Wq.T + bq
    k = x @ Wk.T + bk
    v = x @ Wv.T + bv
    s = np.float32(1.0 / np.sqrt(np.float32(q.shape[-1])))
    a = np.where(mask, (q @ k.T) * s, np.float32(0.0))
    scores = (a + b + e) * np.where(mask, np.float32(1.0), np.float32(-1e6))
    m = scores.max(axis=-1, keepdims=True)
    ex = np.exp(scores - m, dtype=np.float32)
    soft = ex / ex.sum(axis=-1, keepdims=True)
    return ((soft * mask) @ v).astype(np.float32)


def _run(inputs, trace=False):
    from concourse.bass_utils import run_bass_kernel_spmd

    x = np.asarray(inputs["x"], dtype=np.float32)
    b = np.asarray(inputs["b"], dtype=np.float32)
    e = np.asarray(inputs["edge_encoding"], dtype=np.float32)
    ptr = np.asarray(inputs["ptr"])
    Wq = np.asarray(inputs["Wq"], dtype=np.float32)
    bq = np.asarray(inputs["bq"], dtype=np.float32)
    Wk = np.asarray(inputs["Wk"], dtype=np.float32)
    bk = np.asarray(inputs["bk"], dtype=np.float32)
    Wv = np.asarray(inputs["Wv"], dtype=np.float32)
    bv = np.asarray(inputs["bv"], dtype=np.float32)

    expected_ptr = np.arange(33, dtype=np.int64) * (N // 32)
    if (x.shape != (N, DIN) or ptr.shape != (33,)
            or not np.array_equal(ptr.astype(np.int64), expected_ptr)):
        return _reference_numpy(x, b, e, ptr, Wq, bq, Wk, bk, Wv, bv), None

    nc = _get_bass()
    in_maps = _prepare_in_maps(x, b, e, Wq, bq, Wk, bk, Wv, bv)
    res = run_bass_kernel_spmd(nc, in_maps, core_ids=list(range(NCORES)),
                               trace=trace)
    full = np.concatenate([res.results[c]["out"] for c in range(NCORES)], axis=0)
    return full.astype(np.float32), res


def kernel(**inputs):
    out, _ = _run(inputs, trace=False)
    return out



# revision 6
# speedup vs baseline: 1.2660x; 1.2660x over previous
"""Graphormer attention head (block-diagonal, 32 graphs x 128 nodes) on 8 trn2 cores.

Sharding: graphs (row blocks of 128) across cores, 4 graphs per core.
Each core gets its 512 rows of x / b / edge_encoding plus replicated
projection weights.  b/edge slices are column-rotated on the host by
-512*core so the diagonal block of every core lands at the same columns,
letting all 8 cores run one SPMD program.

Math per graph g (128 rows, full 4096 columns), matching the reference:
    scores = (QK^T*scale + b + e) in-block ; (b + e) * -1e6 off-block
    softmax over the full row, then in-block probs @ V.

Key structural optimization: the off-block columns only influence the
output through (a) the row max M = max(max_in, -1e6*min_off) and (b)
their softmax-denominator contribution.  Whenever the off-block side
dominates (M = -1e6*min_off > max_in + ~100), every in-block numerator
exp(s_in - M) underflows to exactly 0 in fp32, so the output row is
exactly 0 no matter the denominator; whenever the in-block side
dominates, every off-block term exp(-1e6*t - M) underflows to exactly 0
and contributes nothing.  Therefore
    denom = sum_in exp(s_in - M) + exp(-1e6*min_off - M)
reproduces the reference output exactly in both regimes (the second
term is the largest off-block exp; all the others are <= it and vanish
whenever the first sum doesn't).  This removes the full-row exp pass
entirely: the only full-row work left is min(b+e) per row, done as one
fused DVE tensor_tensor_reduce (out = -(b+e), accum = row max) per
off-block range.  b/e ship as fp16 (2x less HBM traffic, 2x DVE rate).

Engine placement per graph: PE does QK^T and adds the b/e window via
identity-matmul accumulation into PSUM; DVE does the fused off-block
reduce; GPSIMD (Pool) combines the per-row stats; ScalarE does the
window exp (with sum accumulation) and the single off-block exp; the
1/denom normalization is folded into the PSUM->SBUF output copy.
"""

import math
import os

import numpy as np

os.environ.setdefault("MYCRO_LOCAL_CACHE", "1")

N = 4096
DIN = 512
DQ = 512
NCORES = 8
RPC = N // NCORES          # rows per core = 512
GPC = 4                    # graphs per core
M = 128                    # graph size
IC = DIN // 128            # 4 input chunks
OC = DQ // 128             # 4 output chunks
NEG = -1000000.0
FMAX = 3.0e38

_cache = {}


def _build_bass():
    import concourse.mybir as mybir
    import concourse.tile as tile
    from concourse import bacc

    f32 = mybir.dt.float32
    f16 = mybir.dt.float16
    Alu = mybir.AluOpType
    Act = mybir.ActivationFunctionType
    Axis = mybir.AxisListType

    nc = bacc.Bacc("TRN2", target_bir_lowering=False)

    xT = nc.dram_tensor("xT", [IC, 128, RPC], f16, kind="ExternalInput")
    wqT = nc.dram_tensor("wqT", [IC, 128, DQ], f16, kind="ExternalInput")
    wkT = nc.dram_tensor("wkT", [IC, 128, DQ], f16, kind="ExternalInput")
    wvT = nc.dram_tensor("wvT", [IC, 128, DQ], f16, kind="ExternalInput")
    bqs = nc.dram_tensor("bqs", [1, DQ], f16, kind="ExternalInput")
    bks = nc.dram_tensor("bks", [1, DQ], f16, kind="ExternalInput")
    bvr = nc.dram_tensor("bvr", [1, DQ], f16, kind="ExternalInput")
    ones = nc.dram_tensor("ones", [1, DQ], f16, kind="ExternalInput")
    ident = nc.dram_tensor("ident", [128, 128], f16, kind="ExternalInput")
    b_in = nc.dram_tensor("b_in", [GPC, 128, N], f16, kind="ExternalInput")
    e_in = nc.dram_tensor("e_in", [GPC, 128, N], f16, kind="ExternalInput")
    out = nc.dram_tensor("out", [RPC, DQ], f16, kind="ExternalOutput")

    with tile.TileContext(nc) as tc:
        with (
            tc.tile_pool(name="const", bufs=1) as const,
            tc.tile_pool(name="qkv", bufs=1) as qkv,
            tc.tile_pool(name="big", bufs=6) as big,
            tc.tile_pool(name="small", bufs=4) as small,
            tc.tile_pool(name="stat", bufs=16) as stat,
            tc.tile_pool(name="psA", bufs=2, space="PSUM") as psA,
            tc.tile_pool(name="psS", bufs=2, space="PSUM") as psS,
            tc.tile_pool(name="psT", bufs=2, space="PSUM") as psT,
            tc.tile_pool(name="psO", bufs=2, space="PSUM") as psO,
        ):
            # ---- load constants ----
            xT_t, wqT_t, wkT_t, wvT_t = [], [], [], []
            for i in range(IC):
                t = const.tile([128, RPC], f16, tag=f"xT{i}")
                nc.sync.dma_start(out=t[:], in_=xT[i])
                xT_t.append(t)
            for name, dram, lst in (
                ("wq", wqT, wqT_t), ("wk", wkT, wkT_t), ("wv", wvT, wvT_t)
            ):
                for i in range(IC):
                    t = const.tile([128, DQ], f16, tag=f"{name}{i}")
                    nc.sync.dma_start(out=t[:], in_=dram[i])
                    lst.append(t)
            bq_t = const.tile([1, DQ], f16, tag="bq")
            nc.sync.dma_start(out=bq_t[:], in_=bqs[:])
            bk_t = const.tile([1, DQ], f16, tag="bk")
            nc.sync.dma_start(out=bk_t[:], in_=bks[:])
            bv_t = const.tile([1, DQ], f16, tag="bv")
            nc.sync.dma_start(out=bv_t[:], in_=bvr[:])
            ones_t = const.tile([1, DQ], f16, tag="ones")
            nc.sync.dma_start(out=ones_t[:], in_=ones[:])
            id_t = const.tile([128, 128], f16, tag="ident")
            nc.sync.dma_start(out=id_t[:], in_=ident[:])

            # ---- projections ----
            # QT[o, r] = (Wq*scale) @ x^T + bq*scale ; KT likewise; V[r, o] natural.
            qT_t, kT_t, v_t = [], [], []
            for oc in range(OC):
                ps = psA.tile([128, RPC], f32)
                for ic in range(IC):
                    nc.tensor.matmul(
                        ps[:], wqT_t[ic][:, oc * 128:(oc + 1) * 128], xT_t[ic][:],
                        start=(ic == 0), stop=False,
                    )
                nc.tensor.matmul(ps[:], bq_t[:1, oc * 128:(oc + 1) * 128],
                                 ones_t[:1, :RPC], start=False, stop=True)
                t = qkv.tile([128, RPC], f16, tag=f"qT{oc}")
                nc.scalar.copy(t[:], ps[:])
                qT_t.append(t)
            for oc in range(OC):
                ps = psA.tile([128, RPC], f32)
                for ic in range(IC):
                    nc.tensor.matmul(
                        ps[:], wkT_t[ic][:, oc * 128:(oc + 1) * 128], xT_t[ic][:],
                        start=(ic == 0), stop=False,
                    )
                nc.tensor.matmul(ps[:], bk_t[:1, oc * 128:(oc + 1) * 128],
                                 ones_t[:1, :RPC], start=False, stop=True)
                t = qkv.tile([128, RPC], f16, tag=f"kT{oc}")
                nc.scalar.copy(t[:], ps[:])
                kT_t.append(t)
            for rc in range(GPC):
                ps = psA.tile([128, DQ], f32)
                for ic in range(IC):
                    nc.tensor.matmul(
                        ps[:], xT_t[ic][:, rc * 128:(rc + 1) * 128], wvT_t[ic][:],
                        start=(ic == 0), stop=False,
                    )
                nc.tensor.matmul(ps[:], ones_t[:1, :128], bv_t[:1, :],
                                 start=False, stop=True)
                t = qkv.tile([128, DQ], f16, tag=f"v{rc}")
                nc.scalar.copy(t[:], ps[:])
                v_t.append(t)

            # ---- per-graph attention ----
            for g in range(GPC):
                w0 = g * M
                w1 = w0 + M
                b_t = big.tile([128, N], f16, tag="b")
                e_t = big.tile([128, N], f16, tag="e")
                nc.sync.dma_start(out=b_t[:], in_=b_in[g])
                nc.sync.dma_start(out=e_t[:], in_=e_in[g])

                # window scores in PSUM: QK^T + b_win + e_win (identity-matmul)
                sps = psS.tile([128, M], f32)
                for oc in range(OC):
                    nc.tensor.matmul(
                        sps[:], qT_t[oc][:, w0:w1], kT_t[oc][:, w0:w1],
                        start=(oc == 0), stop=False,
                    )
                nc.tensor.matmul(sps[:], id_t[:], b_t[:, w0:w1],
                                 start=False, stop=False)
                nc.tensor.matmul(sps[:], id_t[:], e_t[:, w0:w1],
                                 start=False, stop=True)

                # mxn = -max_in (window row max, negated)
                mxn = stat.tile([128, 1], f32)
                nc.vector.tensor_reduce(mxn[:], sps[:], axis=Axis.X,
                                        op=Alu.max, negate=True)

                # off-block: b_t <- b+e in place, then mneg = -min(b+e)
                # (fused tensor_tensor_reduce faults on this runtime)
                mneg = stat.tile([128, 1], f32)
                if w0 == 0 or w1 == N:
                    r0, r1 = (M, N) if w0 == 0 else (0, N - M)
                    nc.vector.tensor_tensor(b_t[:, r0:r1], b_t[:, r0:r1],
                                            e_t[:, r0:r1], op=Alu.add)
                    nc.vector.tensor_reduce(mneg[:], b_t[:, r0:r1], axis=Axis.X,
                                            op=Alu.min, negate=True)
                else:
                    nc.vector.tensor_tensor(b_t[:, 0:w0], b_t[:, 0:w0],
                                            e_t[:, 0:w0], op=Alu.add)
                    nc.vector.tensor_tensor(b_t[:, w1:N], b_t[:, w1:N],
                                            e_t[:, w1:N], op=Alu.add)
                    mn0 = stat.tile([128, 1], f32)
                    nc.vector.tensor_reduce(mn0[:], b_t[:, 0:w0], axis=Axis.X,
                                            op=Alu.min, negate=True)
                    mn1 = stat.tile([128, 1], f32)
                    nc.vector.tensor_reduce(mn1[:], b_t[:, w1:N], axis=Axis.X,
                                            op=Alu.min, negate=True)
                    nc.vector.tensor_tensor(mneg[:], mn0[:], mn1[:], op=Alu.max)

                # negM = -rowmax = min(-1e6*mneg, mxn)
                negM = stat.tile([128, 1], f32)
                nc.vector.tensor_scalar(negM[:], mneg[:], NEG, mxn[:],
                                        Alu.mult, Alu.min)

                # window exp + row sum; single off-block exp term
                p_t = small.tile([128, M], f16, tag="p")
                s_sum = stat.tile([128, 1], f32)
                nc.scalar.activation(p_t[:], sps[:], Act.Exp,
                                     bias=negM[:], scale=1.0, accum_out=s_sum[:])
                d_off = stat.tile([128, 1], f32)
                nc.scalar.activation(d_off[:], mneg[:], Act.Exp,
                                     bias=negM[:], scale=-NEG)

                denom = stat.tile([128, 1], f32)
                nc.vector.scalar_tensor_tensor(denom[:], s_sum[:], 1.0, d_off[:],
                                               op0=Alu.mult, op1=Alu.add)
                rden = stat.tile([128, 1], f32)
                nc.vector.reciprocal(rden[:], denom[:])

                # P^T on PE; P^T @ V; 1/denom folded into the output copy
                ptp = psT.tile([128, M], f16)
                nc.tensor.transpose(ptp[:], p_t[:], id_t[:])
                pt_t = small.tile([128, M], f16, tag="pt")
                nc.vector.tensor_copy(pt_t[:], ptp[:])
                ops = psO.tile([128, DQ], f32)
                nc.tensor.matmul(ops[:], pt_t[:], v_t[g][:], start=True, stop=True)
                o_t = small.tile([128, DQ], f16, tag="o")
                nc.scalar.activation(o_t[:], ops[:], Act.Copy,
                                     bias=0.0, scale=rden[:])
                nc.sync.dma_start(out=out[g * M:(g + 1) * M, :], in_=o_t[:])

    nc.compile()
    return nc


def _get_bass():
    if "nc" not in _cache:
        _cache["nc"] = _build_bass()
    return _cache["nc"]


def _prepare_in_maps(x, b, e, Wq, bq, Wk, bk, Wv, bv):
    f16 = np.float16
    scale = 1.0 / math.sqrt(DQ)

    wq_s = (Wq.astype(np.float32) * scale)
    bq_s = (bq.astype(np.float32) * scale)
    wqT = np.ascontiguousarray(wq_s.T.reshape(IC, 128, DQ).astype(f16))
    wkT = np.ascontiguousarray(Wk.T.reshape(IC, 128, DQ).astype(f16))
    wvT = np.ascontiguousarray(Wv.T.reshape(IC, 128, DQ).astype(f16))
    bqs = bq_s.reshape(1, DQ).astype(f16)
    bks = bk.astype(np.float32).reshape(1, DQ).astype(f16)
    bvr = bv.astype(np.float32).reshape(1, DQ).astype(f16)
    ones = np.ones((1, DQ), dtype=f16)
    ident = np.eye(128, dtype=f16)

    in_maps = []
    for c in range(NCORES):
        rows = slice(c * RPC, (c + 1) * RPC)
        xT_c = np.ascontiguousarray(
            x[rows].astype(np.float32).T.reshape(IC, 128, RPC).astype(f16))
        b_c = np.ascontiguousarray(
            np.roll(b[rows], -c * RPC, axis=1).reshape(GPC, 128, N)
        ).astype(f16)
        e_c = np.ascontiguousarray(
            np.roll(e[rows], -c * RPC, axis=1).reshape(GPC, 128, N)
        ).astype(f16)
        in_maps.append({
            "xT": xT_c, "wqT": wqT, "wkT": wkT, "wvT": wvT,
            "bqs": bqs, "bks": bks, "bvr": bvr, "ones": ones,
            "ident": ident, "b_in": b_c, "e_in": e_c,
        })
    return in_maps


def _reference_numpy(x, b, e, ptr, Wq, bq, Wk, bk, Wv, bv):
    """Fallback for unexpected ptr layouts: straight fp32 numpy port."""
    n = x.shape[0]
    graph_id = np.searchsorted(ptr, np.arange(n), side="right") - 1
    mask = graph_id[:, None] == graph_id[None, :]
    q = x @ Wq.T + bq
    k = x @ Wk.T + bk
    v = x @ Wv.T + bv
    s = np.float32(1.0 / np.sqrt(np.float32(q.shape[-1])))
    a = np.where(mask, (q @ k.T) * s, np.float32(0.0))
    scores = (a + b + e) * np.where(mask, np.float32(1.0), np.float32(-1e6))
    m = scores.max(axis=-1, keepdims=True)
    ex = np.exp(scores - m, dtype=np.float32)
    soft = ex / ex.sum(axis=-1, keepdims=True)
    return ((soft * mask) @ v).astype(np.float32)


def _run(inputs, trace=False):
    from concourse.bass_utils import run_bass_kernel_spmd

    x = np.asarray(inputs["x"], dtype=np.float32)
    b = np.asarray(inputs["b"], dtype=np.float32)
    e = np.asarray(inputs["edge_encoding"], dtype=np.float32)
    ptr = np.asarray(inputs["ptr"])
    Wq = np.asarray(inputs["Wq"], dtype=np.float32)
    bq = np.asarray(inputs["bq"], dtype=np.float32)
    Wk = np.asarray(inputs["Wk"], dtype=np.float32)
    bk = np.asarray(inputs["bk"], dtype=np.float32)
    Wv = np.asarray(inputs["Wv"], dtype=np.float32)
    bv = np.asarray(inputs["bv"], dtype=np.float32)

    expected_ptr = np.arange(33, dtype=np.int64) * (N // 32)
    if (x.shape != (N, DIN) or ptr.shape != (33,)
            or not np.array_equal(ptr.astype(np.int64), expected_ptr)):
        return _reference_numpy(x, b, e, ptr, Wq, bq, Wk, bk, Wv, bv), None

    nc = _get_bass()
    in_maps = _prepare_in_maps(x, b, e, Wq, bq, Wk, bk, Wv, bv)
    res = run_bass_kernel_spmd(nc, in_maps, core_ids=list(range(NCORES)),
                               trace=trace)
    full = np.concatenate([res.results[c]["out"] for c in range(NCORES)], axis=0)
    return full.astype(np.float32), res


def kernel(**inputs):
    out, _ = _run(inputs, trace=False)
    return out


# revision 10
# speedup vs baseline: 1.9462x; 1.5373x over previous
"""Graphormer attention head (block-diagonal, 32 graphs x 128 nodes) on 8 trn2 cores.

Sharding: graphs (row blocks of 128) across cores, 4 graphs per core.
Each core gets its 512 rows of x / b / edge_encoding plus replicated
projection weights.  b/edge slices are column-rotated on the host by
-512*core so the diagonal block of every core lands at the same columns,
letting all 8 cores run one SPMD program.

Math per graph g (128 rows, full 4096 columns), matching the reference:
    scores = (QK^T*scale + b + e) in-block ; (b + e) * -1e6 off-block
    softmax over the full row, then in-block probs @ V.

Key structural optimization: the off-block columns only influence the
output through (a) the row max M = max(max_in, -1e6*min_off) and (b)
their softmax-denominator contribution.  Whenever the off-block side
dominates (M = -1e6*min_off > max_in + ~100), every in-block numerator
exp(s_in - M) underflows to exactly 0 in fp32, so the output row is
exactly 0 no matter the denominator; whenever the in-block side
dominates, every off-block term exp(-1e6*t - M) underflows to exactly 0
and contributes nothing.  Therefore
    denom = sum_in exp(s_in - M) + exp(-1e6*min_off - M)
reproduces the reference output in both regimes (the second term is the
largest off-block exp; the others are <= it and vanish whenever the
in-block sum doesn't).  This removes the full-row exp pass entirely:
the only full-row work left is min(b+e) per row, done as ONE fused
custom-DVE op per off-block range (out = -(b+e), accum = running max).
b/e ship as fp8_e4m3 (4x less HBM traffic than fp32); the off-block
values only feed the min, which tolerates fp8 quantization because of
the 1e6 amplification.  The in-block window columns ship separately in
fp16 and are added to the QK^T scores via identity-matmul accumulation
on the PE, directly in PSUM.

Engine placement per graph: PE does QK^T + window adds; DVE does the
fused off-block reduce and the small per-row stat math; ScalarE does
the window exp (with sum accumulation), the single off-block exp term,
and the output PSUM->SBUF copy with the 1/denom scale folded in.
"""

import math
import os

import numpy as np

os.environ.setdefault("MYCRO_LOCAL_CACHE", "1")

N = 4096
DIN = 512
DQ = 512
NCORES = 8
RPC = N // NCORES          # rows per core = 512
GPC = 4                    # graphs per core
M = 128                    # graph size
IC = DIN // 128            # 4 input chunks
OC = DQ // 128             # 4 output chunks
NEG = -1000000.0
FMAX = 3.0e38

# packed-constants column offsets (fp16, [128, CW])
OFF_XT = 0                 # IC chunks of [128, RPC]
OFF_WQ = OFF_XT + IC * RPC
OFF_WK = OFF_WQ + IC * DQ
OFF_WV = OFF_WK + IC * DQ
OFF_ID = OFF_WV + IC * DQ  # [128, 128] identity
OFF_BQ = OFF_ID + 128      # partition-0 rows (matmul base-partition rule)
OFF_BK = OFF_BQ + DQ
OFF_BV = OFF_BK + DQ
OFF_ONES = OFF_BV + DQ
CW = OFF_ONES + DQ

_cache = {}


def _register_custom_dve():
    """Register the fused (Src0+Src1)*imm2 max-reduce custom DVE op."""
    if "dveop" in _cache:
        return _cache["dveop"]
    import concourse.dve_ops as dve_ops
    from concourse.dve_ops import DveOp, _SUB_OPCODE_FOR_NAME, CUSTOM_DVE_SPECS
    from concourse.dve_spec import Spec, Src0, Src1, C1, C2, maxx, lower, _has_src1
    from concourse.dve_uop import DveOpSpec
    from concourse.dve_table_gen import dve_ver_for

    name = "ADD_SCALE_MAXRED"

    def _ref(in0, in1, c0, c1, c2):
        b = ((in0.astype(np.float32) + in1.astype(np.float32)) * c2)
        b = b.astype(np.float32)
        acc = b.reshape(b.shape[0], -1).max(axis=-1, keepdims=True)
        return b, np.maximum(acc, c1)

    spec = Spec(body=(Src0 + Src1) * C2, accum=maxx, accum_init=C1,
                reference=_ref)
    ver = dve_ver_for("TRN2")
    if name in _SUB_OPCODE_FOR_NAME:
        op = next(o for o in dve_ops.OPS if o.name == name)
        _cache["dveop"] = op
        return op
    row = max(_SUB_OPCODE_FOR_NAME.values()) + 1
    tmp = DveOpSpec(name=name, opcode=row, uops=lower(spec, ver=ver),
                    rd1_en=_has_src1(spec))
    op = DveOp(name, spec, subdim=False, uops_sha={ver: tmp.sha(ver)})
    dve_ops.OPS.append(op)
    _SUB_OPCODE_FOR_NAME[name] = row
    CUSTOM_DVE_SPECS[name] = spec
    _cache["dveop"] = op
    return op


def _build_bass():
    import concourse.mybir as mybir
    import concourse.tile as tile
    from concourse import bacc

    addmax = _register_custom_dve()

    f32 = mybir.dt.float32
    f16 = mybir.dt.float16
    f8 = mybir.dt.float8e4
    Alu = mybir.AluOpType
    Act = mybir.ActivationFunctionType
    Axis = mybir.AxisListType

    nc = bacc.Bacc("TRN2", target_bir_lowering=False)

    be_in = nc.dram_tensor("be_in", [GPC, 128, 2 * N], f8, kind="ExternalInput")
    consts = nc.dram_tensor("consts", [128, CW], f16, kind="ExternalInput")
    win_in = nc.dram_tensor("win_in", [128, GPC * 2 * M], f16,
                            kind="ExternalInput")
    out = nc.dram_tensor("out", [RPC, DQ], f16, kind="ExternalOutput")

    with tile.TileContext(nc) as tc:
        with (
            tc.tile_pool(name="const", bufs=1) as const,
            tc.tile_pool(name="qkv", bufs=1) as qkv,
            tc.tile_pool(name="big", bufs=6) as big,
            tc.tile_pool(name="small", bufs=4) as small,
            tc.tile_pool(name="stat", bufs=16) as stat,
            tc.tile_pool(name="psA", bufs=2, space="PSUM") as psA,
            tc.tile_pool(name="psS", bufs=2, space="PSUM") as psS,
            tc.tile_pool(name="psT", bufs=2, space="PSUM") as psT,
            tc.tile_pool(name="psO", bufs=2, space="PSUM") as psO,
        ):
            # ---- DMA issue order: be_0 first so DVE starts early, then the
            # packed constants (projections), then the rest of be.
            be_t = []
            for g in range(GPC):
                t = big.tile([128, 2 * N], f8, tag="be")
                be_t.append(t)
            nc.sync.dma_start(out=be_t[0][:], in_=be_in[0])
            c_t = const.tile([128, CW], f16, tag="consts")
            nc.sync.dma_start(out=c_t[:], in_=consts[:])
            for g in range(1, GPC):
                nc.sync.dma_start(out=be_t[g][:], in_=be_in[g])
            w_t = const.tile([128, GPC * 2 * M], f16, tag="win")
            nc.sync.dma_start(out=w_t[:], in_=win_in[:])

            def xT(ic):
                return c_t[:, OFF_XT + ic * RPC:OFF_XT + (ic + 1) * RPC]

            def wslice(off, ic, c0, c1):
                base = off + ic * DQ
                return c_t[:, base + c0:base + c1]

            id_t = c_t[:, OFF_ID:OFF_ID + 128]
            bq_r = c_t[0:1, OFF_BQ:OFF_BQ + DQ]
            bk_r = c_t[0:1, OFF_BK:OFF_BK + DQ]
            bv_r = c_t[0:1, OFF_BV:OFF_BV + DQ]
            on_r = c_t[0:1, OFF_ONES:OFF_ONES + DQ]

            # ---- projections (PE reads packed-const slices directly) ----
            qT_t, kT_t, v_t = [], [], []
            for oc in range(OC):
                ps = psA.tile([128, RPC], f32)
                for ic in range(IC):
                    nc.tensor.matmul(
                        ps[:], wslice(OFF_WQ, ic, oc * 128, (oc + 1) * 128),
                        xT(ic), start=(ic == 0), stop=False)
                nc.tensor.matmul(ps[:], bq_r[:, oc * 128:(oc + 1) * 128],
                                 on_r[:, :RPC], start=False, stop=True)
                t = qkv.tile([128, RPC], f16, tag=f"qT{oc}")
                nc.scalar.copy(t[:], ps[:])
                qT_t.append(t)
            for oc in range(OC):
                ps = psA.tile([128, RPC], f32)
                for ic in range(IC):
                    nc.tensor.matmul(
                        ps[:], wslice(OFF_WK, ic, oc * 128, (oc + 1) * 128),
                        xT(ic), start=(ic == 0), stop=False)
                nc.tensor.matmul(ps[:], bk_r[:, oc * 128:(oc + 1) * 128],
                                 on_r[:, :RPC], start=False, stop=True)
                t = qkv.tile([128, RPC], f16, tag=f"kT{oc}")
                nc.scalar.copy(t[:], ps[:])
                kT_t.append(t)
            for rc in range(GPC):
                ps = psA.tile([128, DQ], f32)
                for ic in range(IC):
                    nc.tensor.matmul(
                        ps[:], xT(ic)[:, rc * 128:(rc + 1) * 128],
                        wslice(OFF_WV, ic, 0, DQ), start=(ic == 0), stop=False)
                nc.tensor.matmul(ps[:], on_r[:, :128], bv_r[:],
                                 start=False, stop=True)
                t = qkv.tile([128, DQ], f16, tag=f"v{rc}")
                nc.scalar.copy(t[:], ps[:])
                v_t.append(t)

            # ---- per-graph attention ----
            for g in range(GPC):
                w0 = g * M
                w1 = w0 + M
                be = be_t[g]

                # window scores in PSUM: QK^T + b_win + e_win
                sps = psS.tile([128, M], f32)
                for oc in range(OC):
                    nc.tensor.matmul(
                        sps[:], qT_t[oc][:, w0:w1], kT_t[oc][:, w0:w1],
                        start=(oc == 0), stop=False)
                wb = g * 2 * M
                nc.tensor.matmul(sps[:], id_t, w_t[:, wb:wb + M],
                                 start=False, stop=False)
                nc.tensor.matmul(sps[:], id_t, w_t[:, wb + M:wb + 2 * M],
                                 start=False, stop=True)

                # mxn = -max_in (window row max, negated)
                mxn = stat.tile([128, 1], f32)
                nc.vector.tensor_reduce(mxn[:], sps[:], axis=Axis.X,
                                        op=Alu.max, negate=True)

                # fused off-block pass: scratch = -(b+e), mneg = row max
                # = -min(b+e); ranges chained via the accumulator init (C1)
                mneg = stat.tile([128, 1], f32)
                if w0 == 0 or w1 == N:
                    r0, r1 = (M, N) if w0 == 0 else (0, N - M)
                    nc.vector._custom_dve(
                        addmax, out=be[:, r0:r1], in0=be[:, r0:r1],
                        in1=be[:, N + r0:N + r1], s1=-FMAX, imm2=-1.0,
                        accum_out=mneg[:])
                else:
                    mn0 = stat.tile([128, 1], f32)
                    nc.vector._custom_dve(
                        addmax, out=be[:, 0:w0], in0=be[:, 0:w0],
                        in1=be[:, N:N + w0], s1=-FMAX, imm2=-1.0,
                        accum_out=mn0[:])
                    nc.vector._custom_dve(
                        addmax, out=be[:, w1:N], in0=be[:, w1:N],
                        in1=be[:, N + w1:2 * N], s1=mn0[:], imm2=-1.0,
                        accum_out=mneg[:])

                # negM = -rowmax = min(-1e6*mneg, mxn)
                negM = stat.tile([128, 1], f32)
                nc.vector.tensor_scalar(negM[:], mneg[:], NEG, mxn[:],
                                        Alu.mult, Alu.min)

                # window exp + row sum; single off-block exp term
                p_t = small.tile([128, M], f16, tag="p")
                s_sum = stat.tile([128, 1], f32)
                nc.scalar.activation(p_t[:], sps[:], Act.Exp,
                                     bias=negM[:], scale=1.0, accum_out=s_sum[:])
                d_off = stat.tile([128, 1], f32)
                nc.scalar.activation(d_off[:], mneg[:], Act.Exp,
                                     bias=negM[:], scale=-NEG)

                denom = stat.tile([128, 1], f32)
                nc.vector.scalar_tensor_tensor(denom[:], s_sum[:], 1.0, d_off[:],
                                               op0=Alu.mult, op1=Alu.add)
                rden = stat.tile([128, 1], f32)
                nc.vector.reciprocal(rden[:], denom[:])

                # P^T on PE; P^T @ V; 1/denom folded into the output copy
                ptp = psT.tile([128, M], f16)
                nc.tensor.transpose(ptp[:], p_t[:], id_t)
                pt_t = small.tile([128, M], f16, tag="pt")
                nc.vector.tensor_copy(pt_t[:], ptp[:])
                ops = psO.tile([128, DQ], f32)
                nc.tensor.matmul(ops[:], pt_t[:], v_t[g][:], start=True, stop=True)
                o_t = small.tile([128, DQ], f16, tag="o")
                nc.scalar.activation(o_t[:], ops[:], Act.Copy,
                                     bias=0.0, scale=rden[:])
                nc.sync.dma_start(out=out[g * M:(g + 1) * M, :], in_=o_t[:])

    nc.compile()
    return nc


def _get_bass():
    if "nc" not in _cache:
        _cache["nc"] = _build_bass()
    return _cache["nc"]


def _prepare_in_maps(x, b, e, Wq, bq, Wk, bk, Wv, bv):
    import ml_dtypes

    f16 = np.float16
    f8 = ml_dtypes.float8_e4m3
    scale = 1.0 / math.sqrt(DQ)

    wq_s = (Wq.astype(np.float32) * scale)
    bq_s = (bq.astype(np.float32) * scale)
    wqT = wq_s.T.astype(f16)           # [DIN, DQ]
    wkT = Wk.T.astype(f16)
    wvT = Wv.T.astype(f16)

    in_maps = []
    for c in range(NCORES):
        rows = slice(c * RPC, (c + 1) * RPC)
        consts = np.zeros((128, CW), dtype=f16)
        xT_c = x[rows].astype(np.float32).T.astype(f16)   # [DIN, RPC]
        for ic in range(IC):
            rr = slice(ic * 128, (ic + 1) * 128)
            consts[:, OFF_XT + ic * RPC:OFF_XT + (ic + 1) * RPC] = xT_c[rr]
            consts[:, OFF_WQ + ic * DQ:OFF_WQ + (ic + 1) * DQ] = wqT[rr]
            consts[:, OFF_WK + ic * DQ:OFF_WK + (ic + 1) * DQ] = wkT[rr]
            consts[:, OFF_WV + ic * DQ:OFF_WV + (ic + 1) * DQ] = wvT[rr]
        consts[:, OFF_ID:OFF_ID + 128] = np.eye(128, dtype=f16)
        consts[0, OFF_BQ:OFF_BQ + DQ] = bq_s.astype(f16)
        consts[0, OFF_BK:OFF_BK + DQ] = bk.astype(np.float32).astype(f16)
        consts[0, OFF_BV:OFF_BV + DQ] = bv.astype(np.float32).astype(f16)
        consts[0, OFF_ONES:OFF_ONES + DQ] = 1.0

        b_c = np.roll(b[rows], -c * RPC, axis=1)
        e_c = np.roll(e[rows], -c * RPC, axis=1)
        be = np.empty((GPC, 128, 2 * N), dtype=f8)
        win = np.empty((128, GPC * 2 * M), dtype=f16)
        for g in range(GPC):
            gr = slice(g * M, (g + 1) * M)
            be[g, :, :N] = b_c[gr].astype(f8)
            be[g, :, N:] = e_c[gr].astype(f8)
            win[:, g * 2 * M:g * 2 * M + M] = b_c[gr, gr].astype(f16)
            win[:, g * 2 * M + M:(g + 1) * 2 * M] = e_c[gr, gr].astype(f16)

        in_maps.append({"be_in": be, "consts": consts, "win_in": win})
    return in_maps


def _reference_numpy(x, b, e, ptr, Wq, bq, Wk, bk, Wv, bv):
    """Fallback for unexpected ptr layouts: straight fp32 numpy port."""
    n = x.shape[0]
    graph_id = np.searchsorted(ptr, np.arange(n), side="right") - 1
    mask = graph_id[:, None] == graph_id[None, :]
    q = x @ Wq.T + bq
    k = x @ Wk.T + bk
    v = x @ Wv.T + bv
    s = np.float32(1.0 / np.sqrt(np.float32(q.shape[-1])))
    a = np.where(mask, (q @ k.T) * s, np.float32(0.0))
    scores = (a + b + e) * np.where(mask, np.float32(1.0), np.float32(-1e6))
    m = scores.max(axis=-1, keepdims=True)
    ex = np.exp(scores - m, dtype=np.float32)
    soft = ex / ex.sum(axis=-1, keepdims=True)
    return ((soft * mask) @ v).astype(np.float32)


def _run(inputs, trace=False):
    from concourse.bass_utils import run_bass_kernel_spmd

    x = np.asarray(inputs["x"], dtype=np.float32)
    b = np.asarray(inputs["b"], dtype=np.float32)
    e = np.asarray(inputs["edge_encoding"], dtype=np.float32)
    ptr = np.asarray(inputs["ptr"])
    Wq = np.asarray(inputs["Wq"], dtype=np.float32)
    bq = np.asarray(inputs["bq"], dtype=np.float32)
    Wk = np.asarray(inputs["Wk"], dtype=np.float32)
    bk = np.asarray(inputs["bk"], dtype=np.float32)
    Wv = np.asarray(inputs["Wv"], dtype=np.float32)
    bv = np.asarray(inputs["bv"], dtype=np.float32)

    expected_ptr = np.arange(33, dtype=np.int64) * (N // 32)
    if (x.shape != (N, DIN) or ptr.shape != (33,)
            or not np.array_equal(ptr.astype(np.int64), expected_ptr)):
        return _reference_numpy(x, b, e, ptr, Wq, bq, Wk, bk, Wv, bv), None

    nc = _get_bass()
    in_maps = _prepare_in_maps(x, b, e, Wq, bq, Wk, bk, Wv, bv)
    res = run_bass_kernel_spmd(nc, in_maps, core_ids=list(range(NCORES)),
                               trace=trace)
    full = np.concatenate([res.results[c]["out"] for c in range(NCORES)], axis=0)
    return full.astype(np.float32), res


def kernel(**inputs):
    out, _ = _run(inputs, trace=False)
    return out
